# revision 10
# baseline (speedup 1.0000x reference)
"""HGCN (hyperbolic GCN) 2-layer forward for Trainium2, 8 NeuronCores.

Strategy (graph-parallel, dense-spmm, ReduceScatter):
  - Nodes padded 10000 -> 10240 (80 tiles of 128). Ownership is
    chunk-interleaved to match ReduceScatter rank slices: core c owns
    global tiles T(c) = {16p + 2c + i : p in 0..4, i in 0..1}.
  - segment_sum is a dense matmul agg = A @ xt with A[dst, src] built on
    the host. Each core holds the k-slice of A for its OWN 1280 src
    nodes: no feature gather is needed before the matmul. Each core
    computes partial_agg[dst, :] over all 10240 dst using only local
    xt, then a chunked ReduceScatter(add) sums partials across cores
    and hands each core the rows it owns. Communication is off the
    critical path (overlaps the next chunk's matmuls); HypAct runs
    per-chunk as RS results land, overlapping the remaining spmm.
  - A is streamed chunk-by-chunk ([128, 10, 2048] bf16, double
    buffered, one 40KB-per-partition contiguous DMA per chunk).
  - All per-node scalar chains are batched as [128, k] column arrays.

kernel(**inputs) takes the FULL unsharded inputs and returns [2, N, D].
"""

import sys

import numpy as np

for _p in ("/opt/trn_rl_repo",):
    if _p not in sys.path:
        sys.path.append(_p)

import concourse.bass as bass  # noqa: E402
import concourse.tile as tile  # noqa: E402
from concourse import bacc, mybir  # noqa: E402
from concourse.bass_utils import run_bass_kernel_spmd  # noqa: E402
from concourse.masks import make_identity  # noqa: E402

AF = mybir.ActivationFunctionType
ALU = mybir.AluOpType
F32 = mybir.dt.float32
BF16 = mybir.dt.bfloat16

NCORES = 8
N = 10000
D = 256
NP = 10240
TILES = NP // 128       # 80 global node tiles
NT = TILES // NCORES    # 10 tiles owned per core
NCH = 5                 # ReduceScatter chunks per layer
CT = NT // NCH          # own tiles per chunk (2)
CHT = CT * NCORES       # global tiles per chunk (16)
CHC = CHT * 128         # dst columns per chunk (2048)
MAXN = 1.0 - 4e-3       # PROJ_EPS clip for c=1
MINN = 1e-15


def own_tiles(c):
    """Global tile indices owned by core c, in local order."""
    return [p * CHT + c * CT + i for p in range(NCH) for i in range(CT)]


def _mm_np_dtype():
    import ml_dtypes

    return np.dtype(ml_dtypes.bfloat16)


def build_nc(y2s):
    """Build the per-core Bass program. y2s = (||hyp_b1||^2, ||hyp_b2||^2)."""
    nc = bacc.Bacc("TRN2", target_bir_lowering=False, debug=False,
                   num_devices=NCORES)

    xc = nc.dram_tensor("xc", [NT, 128, D], F32, kind="ExternalInput")
    # A k-slice, partition-major per chunk: a[p][s][j] = row of src j*128+s
    # (local tile j), dst cols [p*2048, (p+1)*2048)
    a_d = nc.dram_tensor("a", [NCH, 128, NT, CHC], BF16, kind="ExternalInput")
    w1t = nc.dram_tensor("w1t", [2, 128, D], F32, kind="ExternalInput")
    w2t = nc.dram_tensor("w2t", [2, 128, D], F32, kind="ExternalInput")
    hb1 = nc.dram_tensor("hb1", [128, D], F32, kind="ExternalInput")
    hb2 = nc.dram_tensor("hb2", [128, D], F32, kind="ExternalInput")
    e1_d = nc.dram_tensor("e1", [NT, 128, D], F32, kind="ExternalOutput")
    e2_d = nc.dram_tensor("e2", [NT, 128, D], F32, kind="ExternalOutput")

    groups = [list(range(NCORES))]

    with tile.TileContext(nc) as tc:
        with (
            tc.tile_pool(name="const", bufs=1) as const,
            tc.tile_pool(name="persist", bufs=1) as persist,
            tc.tile_pool(name="sqp", bufs=3) as sqp,
            tc.tile_pool(name="htp", bufs=4) as htp,
            tc.tile_pool(name="ablk", bufs=2) as ablk,
            tc.tile_pool(name="stg", bufs=4) as stg,
            tc.tile_pool(name="aggp", bufs=4) as aggp,
            tc.tile_pool(name="pst", bufs=2, space="PSUM") as pst,
            tc.tile_pool(name="psmx", bufs=1, space="PSUM") as psmx,
            tc.tile_pool(name="psp", bufs=4, space="PSUM") as psp,
            tc.tile_pool(name="dram", bufs=1, space="DRAM") as dram,
        ):
            ident = const.tile([128, 128], F32, name="ident")
            make_identity(nc, ident)

            w_sb = []
            for li, wd in enumerate((w1t, w2t)):
                w = const.tile([128, 2, D], F32, name=f"w{li}")
                nc.sync.dma_start(w[:], wd.ap().rearrange("k p n -> p k n"))
                w_sb.append(w)
            hb_sb = []
            for li, hd in enumerate((hb1, hb2)):
                h = const.tile([128, D], F32, name=f"hb{li}")
                nc.sync.dma_start(h[:], hd.ap())
                hb_sb.append(h)

            def sc(name, k=NT):
                return persist.tile([128, k], F32, name=name)

            def square_accum(src_ap, accum_ap):
                s = sqp.tile([128, D], F32, name="sqt", tag="sqt")
                nc.scalar.activation(s[:], src_ap, AF.Square, accum_out=accum_ap)

            def clamp_recip(dst, src, name, k=NT):
                c = sc(name + "_c", k)
                nc.vector.tensor_scalar_max(c[:], src[:], MINN)
                nc.vector.reciprocal(dst[:], c[:])

            # All ACT transcendentals are expressed with {Exp, Ln, Square,
            # Copy, Identity} only, which share ONE activation table
            # (natural_log_exp_and_others) -- any mix of Sqrt/Tanh/Ln would
            # pay a 1.3us table reload per function switch.
            def sqrt_act(dst, x, name, k=NT):
                """dst = sqrt(max(x, 1e-30)) via exp(0.5*ln(x))."""
                c = sc(name + "_sc", k)
                nc.vector.tensor_scalar_max(c[:], x[:], 1e-30)
                l = sc(name + "_sl", k)
                nc.scalar.activation(l[:], c[:], AF.Ln)
                nc.scalar.activation(dst[:], l[:], AF.Exp, scale=0.5)

            def tanh_act(dst, x, name, k=NT, pre=1.0):
                """dst = tanh(pre*x) for x >= 0, via u = exp(-2*pre*x),
                tanh = (1-u)/(1+u)."""
                u = sc(name + "_tu", k)
                nc.scalar.activation(u[:], x[:], AF.Exp, scale=-2.0 * pre)
                num = sc(name + "_tn", k)
                nc.vector.tensor_scalar(num[:], u[:], -1.0, 1.0,
                                        ALU.mult, ALU.add)
                den = sc(name + "_td", k)
                nc.vector.tensor_scalar_add(den[:], u[:], 1.0)
                rden = sc(name + "_tr", k)
                nc.vector.reciprocal(rden[:], den[:])
                nc.vector.tensor_tensor(dst[:], num[:], rden[:], ALU.mult)

            def artanh_ln(dst, x, name, k=NT):
                """dst = ln((1+x)/(1-x)); caller owns the 0.5 factor."""
                ap1 = sc(name + "_ap", k)
                am1 = sc(name + "_am", k)
                ram = sc(name + "_ram", k)
                q = sc(name + "_q", k)
                nc.scalar.activation(ap1[:], x[:], AF.Identity, bias=1.0)
                nc.scalar.activation(am1[:], x[:], AF.Identity, bias=1.0, scale=-1.0)
                nc.vector.reciprocal(ram[:], am1[:])
                nc.vector.tensor_tensor(q[:], ap1[:], ram[:], ALU.mult)
                nc.scalar.activation(dst[:], q[:], AF.Ln)

            # ---------------- encode: h = proj(expmap0(x)) ----------------
            x_sb = persist.tile([128, NT, D], F32, name="x_sb", tag="bigA")
            nc.sync.dma_start(x_sb[:], xc.ap().rearrange("t p d -> p t d"))
            h_all = persist.tile([128, NT, D], F32, name="h_all", tag="bigB")
            xn2 = sc("xn2")
            for t in range(NT):
                square_accum(x_sb[:, t, :], xn2[:, t : t + 1])
            un = sc("un")
            sqrt_act(un, xn2, "enc_un")
            run_ = sc("run")
            clamp_recip(run_, un, "enc_r")
            thx = sc("thx")
            tanh_act(thx, un, "enc_th")
            mn0 = sc("mn0")
            nc.vector.tensor_scalar_min(mn0[:], thx[:], MAXN)
            s0 = sc("s0")
            nc.vector.tensor_tensor(s0[:], mn0[:], run_[:], ALU.mult)
            for t in range(NT):
                nc.vector.tensor_scalar_mul(h_all[:, t, :], x_sb[:, t, :],
                                            s0[:, t : t + 1])

            def hyp_linear(li, h_in, hnorm):
                """HypLinear + logmap0: h_in [128,NT,D] on-ball, hnorm [128,NT]
                row norms. Returns xt_all [128,NT,D] bf16 tangent features."""
                L = f"l{li}_"
                w = w_sb[li]
                hb = hb_sb[li]
                y2 = float(y2s[li])

                mx_all = persist.tile([128, NT, D], F32, name=L + "mx", tag="bigA")
                mn2 = sc(L + "mn2")
                for t in range(NT):
                    hT = htp.tile([128, 2, 128], F32, name="hT", tag="hT")
                    for kc in range(2):
                        psT = pst.tile([128, 128], F32, name="psT", tag="psT")
                        nc.tensor.transpose(
                            psT[:], h_in[:, t, kc * 128 : (kc + 1) * 128], ident[:])
                        nc.vector.tensor_copy(hT[:, kc, :], psT[:])
                    pmx = psmx.tile([128, D], F32, name="pmx", tag="pmx")
                    nc.tensor.matmul(pmx[:], hT[:, 0, :], w[:, 0, :],
                                     start=True, stop=False)
                    nc.tensor.matmul(pmx[:], hT[:, 1, :], w[:, 1, :],
                                     start=False, stop=True)
                    square_accum(pmx[:], mn2[:, t : t + 1])
                    nc.vector.tensor_copy(mx_all[:, t, :], pmx[:])

                mxn = sc(L + "mxn")
                sqrt_act(mxn, mn2, L + "mxn")
                nc.vector.tensor_scalar_max(mxn[:], mxn[:], MINN)
                rxn = sc(L + "rxn")
                clamp_recip(rxn, hnorm, L + "rxn")
                rmxn = sc(L + "rmxn")
                nc.vector.reciprocal(rmxn[:], mxn[:])
                atx = sc(L + "atx")
                artanh_ln(atx, hnorm, L + "atx")
                targ = sc(L + "targ")
                nc.vector.tensor_tensor(targ[:], mxn[:], rxn[:], ALU.mult)
                nc.vector.tensor_tensor(targ[:], targ[:], atx[:], ALU.mult)
                th = sc(L + "th")
                tanh_act(th, targ, L + "th", pre=0.5)
                sres = sc(L + "sres")
                nc.vector.tensor_tensor(sres[:], th[:], rmxn[:], ALU.mult)
                rth = sc(L + "rth")
                clamp_recip(rth, th, L + "rth")
                f1 = sc(L + "f1")
                nc.vector.tensor_scalar(f1[:], rth[:], MAXN, 1.0, ALU.mult, ALU.min)
                nres = sc(L + "nres")
                nc.vector.tensor_scalar_min(nres[:], th[:], MAXN)
                x2 = sc(L + "x2")
                nc.vector.tensor_tensor(x2[:], nres[:], nres[:], ALU.mult)

                ryp = sc(L + "ryp")
                for t in range(NT):
                    prod = sqp.tile([128, D], F32, name="prodt", tag="prodt")
                    nc.vector.tensor_tensor(prod[:], mx_all[:, t, :], hb[:],
                                            ALU.mult)
                    nc.vector.reduce_sum(ryp[:, t : t + 1], prod[:],
                                         axis=mybir.AxisListType.X)

                xy = sc(L + "xy")
                nc.vector.tensor_tensor(xy[:], ryp[:], sres[:], ALU.mult)
                nc.vector.tensor_tensor(xy[:], xy[:], f1[:], ALU.mult)
                apre = sc(L + "apre")
                nc.vector.tensor_scalar(apre[:], xy[:], 2.0, 1.0 + y2,
                                        ALU.mult, ALU.add)
                alpha = sc(L + "alpha")
                nc.vector.tensor_tensor(alpha[:], apre[:], f1[:], ALU.mult)
                beta = sc(L + "beta")
                nc.scalar.activation(beta[:], x2[:], AF.Identity,
                                     bias=1.0, scale=-1.0)
                den = sc(L + "den")
                nc.vector.tensor_scalar(den[:], x2[:], y2, 1.0, ALU.mult, ALU.add)
                xy2 = sc(L + "xy2")
                nc.vector.tensor_scalar_mul(xy2[:], xy[:], 2.0)
                nc.vector.tensor_tensor(den[:], den[:], xy2[:], ALU.add)
                dinv = sc(L + "dinv")
                clamp_recip(dinv, den, L + "dinv")
                asc = sc(L + "asc")
                nc.vector.tensor_tensor(asc[:], alpha[:], dinv[:], ALU.mult)
                nc.vector.tensor_tensor(asc[:], asc[:], sres[:], ALU.mult)
                bsc = sc(L + "bsc")
                nc.vector.tensor_tensor(bsc[:], beta[:], dinv[:], ALU.mult)

                h2_all = persist.tile([128, NT, D], F32, name=L + "h2", tag="bigB")
                hn2 = sc(L + "hn2")
                for t in range(NT):
                    t1 = sqp.tile([128, D], F32, name="t1t", tag="t1t")
                    nc.vector.tensor_scalar_mul(t1[:], mx_all[:, t, :],
                                                asc[:, t : t + 1])
                    t2 = sqp.tile([128, D], F32, name="t2t", tag="t2t")
                    nc.scalar.activation(t2[:], hb[:], AF.Copy,
                                         scale=bsc[:, t : t + 1])
                    nc.vector.tensor_tensor(h2_all[:, t, :], t1[:], t2[:], ALU.add)
                    square_accum(h2_all[:, t, :], hn2[:, t : t + 1])

                hn = sc(L + "hn")
                sqrt_act(hn, hn2, L + "hn")
                rhn = sc(L + "rhn")
                clamp_recip(rhn, hn, L + "rhn")
                f2 = sc(L + "f2")
                nc.vector.tensor_scalar(f2[:], rhn[:], MAXN, 1.0, ALU.mult, ALU.min)
                m = sc(L + "m")
                nc.vector.tensor_scalar_min(m[:], hn[:], MAXN)
                rm = sc(L + "rm")
                clamp_recip(rm, m, L + "rm")
                atm = sc(L + "atm")
                artanh_ln(atm, m, L + "atm")
                g = sc(L + "g")
                nc.vector.tensor_tensor(g[:], atm[:], rm[:], ALU.mult)
                nc.vector.tensor_tensor(g[:], g[:], f2[:], ALU.mult)
                nc.vector.tensor_scalar_mul(g[:], g[:], 0.5)

                xt_all = persist.tile([128, NT, D], BF16, name=L + "xt",
                                      tag="bigC")
                for t in range(NT):
                    nc.vector.tensor_scalar_mul(xt_all[:, t, :], h2_all[:, t, :],
                                                g[:, t : t + 1])
                return xt_all

            def spmm_layer(li, xt_all, e_out_d):
                """Chunked partial spmm + ReduceScatter + per-chunk HypAct.

                Queue discipline (all engine queues are in-order, so no
                pre-RS work may be emitted after RS-dependent work on the
                same queue): matmuls on tensor; PSUM evacuation on vector
                BEFORE any act-chain ops of the same cycle (act chains are
                emitted LAG chunks behind); A-stream + stage-out DMAs on
                sync (never RS-dependent); RS + its readback on gpsimd;
                e-store issued from vector right after its act chain.
                Returns (e_all [128,NT,D] f32, mm2_all [128,NT] norms)."""
                L = f"l{li}_"
                LAG = 2
                e_all = persist.tile([128, NT, D], F32, name=L + "e", tag="bigE")
                xt2_all = persist.tile([128, NT, D], F32, name=L + "xt2",
                                       tag="bigD")
                mm2_all = sc(L + "mm2")
                aggs = {}

                def hyp_act_chunk(p):
                    P = f"{L}c{p}_"
                    agg = aggs.pop(p)
                    r2 = sc(P + "r2", CT)
                    for i in range(CT):
                        square_accum(agg[:, i, :], r2[:, i : i + 1])
                    rn = sc(P + "rn", CT)
                    sqrt_act(rn, r2, P + "rn2", CT)
                    rrn = sc(P + "rrn", CT)
                    clamp_recip(rrn, rn, P + "rrn", CT)
                    th2 = sc(P + "th2", CT)
                    tanh_act(th2, rn, P + "th2", CT)
                    m1 = sc(P + "m1", CT)
                    nc.vector.tensor_scalar_min(m1[:], th2[:], MAXN)
                    rm1 = sc(P + "rm1", CT)
                    clamp_recip(rm1, m1, P + "rm1", CT)
                    s1 = sc(P + "s1", CT)
                    nc.vector.tensor_tensor(s1[:], m1[:], rrn[:], ALU.mult)
                    atq = sc(P + "atq", CT)
                    artanh_ln(atq, m1, P + "atq", CT)
                    qs = sc(P + "qs", CT)
                    nc.vector.tensor_tensor(qs[:], s1[:], atq[:], ALU.mult)
                    nc.vector.tensor_tensor(qs[:], qs[:], rm1[:], ALU.mult)
                    nc.vector.tensor_scalar_mul(qs[:], qs[:], 0.5)

                    n2b = sc(P + "n2b", CT)
                    for i in range(CT):
                        t = p * CT + i
                        nc.vector.tensor_scalar(xt2_all[:, t, :], agg[:, i, :],
                                                qs[:, i : i + 1], 0.0,
                                                ALU.mult, ALU.max)
                        square_accum(xt2_all[:, t, :], n2b[:, i : i + 1])

                    un2 = sc(P + "un2", CT)
                    sqrt_act(un2, n2b, P + "un2", CT)
                    run2 = sc(P + "run2", CT)
                    clamp_recip(run2, un2, P + "run2", CT)
                    th3 = sc(P + "th3", CT)
                    tanh_act(th3, un2, P + "th3", CT)
                    mm2 = mm2_all[:, p * CT : (p + 1) * CT]
                    nc.vector.tensor_scalar_min(mm2, th3[:], MAXN)
                    ss = sc(P + "ss", CT)
                    nc.vector.tensor_tensor(ss[:], mm2, run2[:], ALU.mult)

                    for i in range(CT):
                        t = p * CT + i
                        nc.vector.tensor_scalar_mul(e_all[:, t, :],
                                                    xt2_all[:, t, :],
                                                    ss[:, i : i + 1])
                    nc.scalar.dma_start(
                        e_out_d.ap()[p * CT : (p + 1) * CT].rearrange(
                            "t p d -> p t d"),
                        e_all[:, p * CT : (p + 1) * CT, :])

                a_tiles = {}
                a_tiles[0] = ablk.tile([128, NT, CHC], BF16, name="a_sb",
                                       tag="a_sb")
                nc.sync.dma_start(a_tiles[0][:], a_d.ap()[0])
                for p in range(NCH):
                    P = f"{L}c{p}_"
                    if p + 1 < NCH:
                        a_tiles[p + 1] = ablk.tile([128, NT, CHC], BF16,
                                                   name="a_sb", tag="a_sb")
                        nc.sync.dma_start(a_tiles[p + 1][:], a_d.ap()[p + 1])
                    a_sb = a_tiles.pop(p)
                    # rs_in is rank-major [NCORES, 128, CT*D]; pair pr of the
                    # chunk IS rank pr's slice (CT=2), written per-pair as
                    # soon as its matmuls finish (contiguous 1KB runs).
                    rs_in = dram.tile([NCORES, 128, CT * D], BF16,
                                      name=P + "rsi", tag=f"rsi{p % 2}")
                    for pr in range(CHT // 2):
                        ps = psp.tile([128, 2, D], F32, name="ps", tag="ps")
                        for kt in range(NT):
                            for h in range(2):
                                nc.tensor.matmul(
                                    ps[:, h, :],
                                    a_sb[:, kt,
                                         pr * 256 + h * 128 : pr * 256 + h * 128 + 128],
                                    xt_all[:, kt, :],
                                    start=(kt == 0 and h == 0),
                                    stop=(kt == NT - 1),
                                    skip_group_check=True)
                        pair = stg.tile([128, CT, D], BF16, name="pair",
                                        tag="pair")
                        nc.vector.tensor_copy(pair[:], ps[:])
                        nc.sync.dma_start(
                            rs_in[:][pr],
                            pair[:].rearrange("p t d -> p (t d)"))
                    rs_out = dram.tile([128, CT * D], BF16, name=P + "rso",
                                       tag=f"rso{p % 2}")
                    nc.gpsimd.collective_compute(
                        "ReduceScatter", ALU.add, replica_groups=groups,
                        ins=[rs_in[:].opt()], outs=[rs_out[:].opt()])
                    agg = aggp.tile([128, CT, D], BF16, name="agg", tag="agg")
                    nc.gpsimd.dma_start(
                        agg[:].rearrange("p t d -> p (t d)"), rs_out[:])
                    aggs[p] = agg
                    if p >= LAG:
                        hyp_act_chunk(p - LAG)
                for p in range(NCH - LAG, NCH):
                    hyp_act_chunk(p)
                return e_all, mm2_all

            xt0 = hyp_linear(0, h_all, mn0)
            e1_all, n1 = spmm_layer(0, xt0, e1_d)
            xt1 = hyp_linear(1, e1_all, n1)
            spmm_layer(1, xt1, e2_d)

    nc.compile()
    return nc


def _hyp_bias(b):
    """proj(expmap0(b, c=1), c=1) in float32, mirroring the reference."""
    b = b.astype(np.float32)
    un = np.maximum(np.sqrt((b * b).sum()), np.float32(MINN)).astype(np.float32)
    h = (np.tanh(un) * b / un).astype(np.float32)
    n = np.maximum(np.sqrt((h * h).sum()), np.float32(MINN)).astype(np.float32)
    if n > np.float32(MAXN):
        h = (h / n * np.float32(MAXN)).astype(np.float32)
    return h


def prepare_inputs(x, W1, b1, W2, b2, edge_index, edge_weight):
    mmnp = _mm_np_dtype()
    x = np.asarray(x, np.float32)
    W1 = np.asarray(W1, np.float32)
    W2 = np.asarray(W2, np.float32)
    b1 = np.asarray(b1, np.float32)
    b2 = np.asarray(b2, np.float32)
    ew = np.asarray(edge_weight, np.float32)
    src = np.asarray(edge_index[0], np.int64)
    dst = np.asarray(edge_index[1], np.int64)

    # A[dst, src]; device matmul computes partial[dst] += A^T[src, dst] rows
    AT = np.zeros((NP, NP), np.float32)
    np.add.at(AT, (src, dst), ew)
    ATb = AT.astype(mmnp)

    xfull = np.zeros((NP, D), np.float32)
    xfull[:N] = x

    hb1 = _hyp_bias(b1)
    hb2 = _hyp_bias(b2)
    y2s = (float((hb1.astype(np.float64) ** 2).sum()),
           float((hb2.astype(np.float64) ** 2).sum()))

    w1t = np.ascontiguousarray(W1.T).reshape(2, 128, D)
    w2t = np.ascontiguousarray(W2.T).reshape(2, 128, D)
    hb1_b = np.tile(hb1[None, :], (128, 1)).astype(np.float32)
    hb2_b = np.tile(hb2[None, :], (128, 1)).astype(np.float32)

    in_maps = []
    for c in range(NCORES):
        tl = own_tiles(c)
        xcr = np.stack([xfull[g * 128 : (g + 1) * 128] for g in tl], 0)
        # a[p, s, j, :] = AT[tl[j]*128 + s, p*CHC:(p+1)*CHC]
        ac = np.empty((NCH, 128, NT, CHC), mmnp)
        for j, g in enumerate(tl):
            rows = ATb[g * 128 : (g + 1) * 128]  # [128, NP]
            for p in range(NCH):
                ac[p, :, j, :] = rows[:, p * CHC : (p + 1) * CHC]
        in_maps.append({
            "xc": np.ascontiguousarray(xcr),
            "a": ac,
            "w1t": w1t, "w2t": w2t,
            "hb1": hb1_b, "hb2": hb2_b,
        })
    return in_maps, y2s


def assemble(results):
    e1 = np.zeros((NP, D), np.float32)
    e2 = np.zeros((NP, D), np.float32)
    for c, r in enumerate(results):
        tl = own_tiles(c)
        r1 = r["e1"].reshape(NT, 128, D)
        r2 = r["e2"].reshape(NT, 128, D)
        for j, g in enumerate(tl):
            e1[g * 128 : (g + 1) * 128] = r1[j]
            e2[g * 128 : (g + 1) * 128] = r2[j]
    return np.stack([e1[:N], e2[:N]], 0).astype(np.float32)


def run(inputs, trace=False):
    in_maps, y2s = prepare_inputs(**inputs)
    nc = build_nc(y2s)
    res = run_bass_kernel_spmd(nc, in_maps, core_ids=list(range(NCORES)),
                               trace=trace)
    return assemble(res.results), res


def kernel(**inputs):
    out, _ = run(inputs, trace=False)
    return out


# revision 12
# speedup vs baseline: 1.0706x; 1.0706x over previous
"""HGCN (hyperbolic GCN) 2-layer forward for Trainium2, 8 NeuronCores.

Strategy (graph-parallel, dense-spmm, ReduceScatter):
  - Nodes padded 10000 -> 10240 (80 tiles of 128). Ownership is
    chunk-interleaved to match ReduceScatter rank slices: core c owns
    global tiles T(c) = {16p + 2c + i : p in 0..4, i in 0..1}.
  - segment_sum is a dense matmul agg = A @ xt with A[dst, src] built on
    the host. Each core holds the k-slice of A for its OWN 1280 src
    nodes: no feature gather is needed before the matmul. Each core
    computes partial_agg[dst, :] over all 10240 dst using only local
    xt, then a chunked ReduceScatter(add) sums partials across cores
    and hands each core the rows it owns. Communication is off the
    critical path (overlaps the next chunk's matmuls); HypAct runs
    per-chunk as RS results land, overlapping the remaining spmm.
  - A is streamed chunk-by-chunk ([128, 10, 2048] bf16, double
    buffered, one 40KB-per-partition contiguous DMA per chunk).
  - All per-node scalar chains are batched as [128, k] column arrays.

kernel(**inputs) takes the FULL unsharded inputs and returns [2, N, D].
"""

import sys

import numpy as np

for _p in ("/opt/trn_rl_repo",):
    if _p not in sys.path:
        sys.path.append(_p)

import concourse.bass as bass  # noqa: E402
import concourse.tile as tile  # noqa: E402
from concourse import bacc, mybir  # noqa: E402
from concourse.bass_utils import run_bass_kernel_spmd  # noqa: E402
from concourse.masks import make_identity  # noqa: E402

AF = mybir.ActivationFunctionType
ALU = mybir.AluOpType
F32 = mybir.dt.float32
BF16 = mybir.dt.bfloat16

NCORES = 8
N = 10000
D = 256
NP = 10240
TILES = NP // 128       # 80 global node tiles
NT = TILES // NCORES    # 10 tiles owned per core
NCH = 5                 # ReduceScatter chunks per layer
CT = NT // NCH          # own tiles per chunk (2)
CHT = CT * NCORES       # global tiles per chunk (16)
CHC = CHT * 128         # dst columns per chunk (2048)
MAXN = 1.0 - 4e-3       # PROJ_EPS clip for c=1
MINN = 1e-15


def own_tiles(c):
    """Global tile indices owned by core c, in local order."""
    return [p * CHT + c * CT + i for p in range(NCH) for i in range(CT)]


def _mm_np_dtype():
    import ml_dtypes

    return np.dtype(ml_dtypes.bfloat16)


def build_nc(y2s):
    """Build the per-core Bass program. y2s = (||hyp_b1||^2, ||hyp_b2||^2)."""
    nc = bacc.Bacc("TRN2", target_bir_lowering=False, debug=False,
                   num_devices=NCORES)

    xc = nc.dram_tensor("xc", [NT, 128, D], F32, kind="ExternalInput")
    # A k-slice, partition-major per chunk: a[p][s][j] = row of src j*128+s
    # (local tile j), dst cols [p*2048, (p+1)*2048)
    a_d = nc.dram_tensor("a", [NCH, 128, NT, CHC], BF16, kind="ExternalInput")
    w1t = nc.dram_tensor("w1t", [2, 128, D], F32, kind="ExternalInput")
    w2t = nc.dram_tensor("w2t", [2, 128, D], F32, kind="ExternalInput")
    hb1 = nc.dram_tensor("hb1", [128, D], F32, kind="ExternalInput")
    hb2 = nc.dram_tensor("hb2", [128, D], F32, kind="ExternalInput")
    e1_d = nc.dram_tensor("e1", [NT, 128, D], F32, kind="ExternalOutput")
    e2_d = nc.dram_tensor("e2", [NT, 128, D], F32, kind="ExternalOutput")

    groups = [list(range(NCORES))]

    with tile.TileContext(nc) as tc:
        with (
            tc.tile_pool(name="const", bufs=1) as const,
            tc.tile_pool(name="persist", bufs=1) as persist,
            tc.tile_pool(name="sqp", bufs=3) as sqp,
            tc.tile_pool(name="htp", bufs=4) as htp,
            tc.tile_pool(name="ablk", bufs=2) as ablk,
            tc.tile_pool(name="stg", bufs=4) as stg,
            tc.tile_pool(name="aggp", bufs=4) as aggp,
            tc.tile_pool(name="pst", bufs=2, space="PSUM") as pst,
            tc.tile_pool(name="psmx", bufs=1, space="PSUM") as psmx,
            tc.tile_pool(name="psp", bufs=4, space="PSUM") as psp,
            tc.tile_pool(name="dram", bufs=1, space="DRAM") as dram,
        ):
            ident = const.tile([128, 128], F32, name="ident")
            make_identity(nc, ident)

            w_sb = []
            for li, wd in enumerate((w1t, w2t)):
                w = const.tile([128, 2, D], F32, name=f"w{li}")
                nc.sync.dma_start(w[:], wd.ap().rearrange("k p n -> p k n"))
                w_sb.append(w)
            hb_sb = []
            for li, hd in enumerate((hb1, hb2)):
                h = const.tile([128, D], F32, name=f"hb{li}")
                nc.sync.dma_start(h[:], hd.ap())
                hb_sb.append(h)

            def sc(name, k=NT):
                return persist.tile([128, k], F32, name=name)

            def square_accum(src_ap, accum_ap):
                s = sqp.tile([128, D], F32, name="sqt", tag="sqt")
                nc.scalar.activation(s[:], src_ap, AF.Square, accum_out=accum_ap)

            def clamp_recip(dst, src, name, k=NT):
                c = sc(name + "_c", k)
                nc.vector.tensor_scalar_max(c[:], src[:], MINN)
                nc.vector.reciprocal(dst[:], c[:])

            # All ACT transcendentals are expressed with {Exp, Ln, Square,
            # Copy, Identity} only, which share ONE activation table
            # (natural_log_exp_and_others) -- any mix of Sqrt/Tanh/Ln would
            # pay a 1.3us table reload per function switch.
            def sqrt_act(dst, x, name, k=NT):
                """dst = sqrt(max(x, 1e-30)) via exp(0.5*ln(x))."""
                c = sc(name + "_sc", k)
                nc.vector.tensor_scalar_max(c[:], x[:], 1e-30)
                l = sc(name + "_sl", k)
                nc.scalar.activation(l[:], c[:], AF.Ln)
                nc.scalar.activation(dst[:], l[:], AF.Exp, scale=0.5)

            def tanh_act(dst, x, name, k=NT, pre=1.0):
                """dst = tanh(pre*x) for x >= 0, via u = exp(-2*pre*x),
                tanh = (1-u)/(1+u)."""
                u = sc(name + "_tu", k)
                nc.scalar.activation(u[:], x[:], AF.Exp, scale=-2.0 * pre)
                num = sc(name + "_tn", k)
                nc.vector.tensor_scalar(num[:], u[:], -1.0, 1.0,
                                        ALU.mult, ALU.add)
                den = sc(name + "_td", k)
                nc.vector.tensor_scalar_add(den[:], u[:], 1.0)
                rden = sc(name + "_tr", k)
                nc.vector.reciprocal(rden[:], den[:])
                nc.vector.tensor_tensor(dst[:], num[:], rden[:], ALU.mult)

            def artanh_ln(dst, x, name, k=NT):
                """dst = ln((1+x)/(1-x)); caller owns the 0.5 factor."""
                ap1 = sc(name + "_ap", k)
                am1 = sc(name + "_am", k)
                ram = sc(name + "_ram", k)
                q = sc(name + "_q", k)
                nc.scalar.activation(ap1[:], x[:], AF.Identity, bias=1.0)
                nc.scalar.activation(am1[:], x[:], AF.Identity, bias=1.0, scale=-1.0)
                nc.vector.reciprocal(ram[:], am1[:])
                nc.vector.tensor_tensor(q[:], ap1[:], ram[:], ALU.mult)
                nc.scalar.activation(dst[:], q[:], AF.Ln)

            # ---------------- encode: h = proj(expmap0(x)) ----------------
            x_sb = persist.tile([128, NT, D], F32, name="x_sb", tag="bigA")
            nc.sync.dma_start(x_sb[:], xc.ap().rearrange("t p d -> p t d"))
            h_all = persist.tile([128, NT, D], F32, name="h_all", tag="bigB")
            xn2 = sc("xn2")
            for t in range(NT):
                square_accum(x_sb[:, t, :], xn2[:, t : t + 1])
            un = sc("un")
            sqrt_act(un, xn2, "enc_un")
            run_ = sc("run")
            clamp_recip(run_, un, "enc_r")
            thx = sc("thx")
            tanh_act(thx, un, "enc_th")
            mn0 = sc("mn0")
            nc.vector.tensor_scalar_min(mn0[:], thx[:], MAXN)
            s0 = sc("s0")
            nc.vector.tensor_tensor(s0[:], mn0[:], run_[:], ALU.mult)
            for t in range(NT):
                nc.vector.tensor_scalar_mul(h_all[:, t, :], x_sb[:, t, :],
                                            s0[:, t : t + 1])

            def hyp_linear(li, h_in, hnorm):
                """HypLinear + logmap0: h_in [128,NT,D] on-ball, hnorm [128,NT]
                row norms. Returns xt_all [128,NT,D] bf16 tangent features."""
                L = f"l{li}_"
                w = w_sb[li]
                hb = hb_sb[li]
                y2 = float(y2s[li])

                mx_all = persist.tile([128, NT, D], F32, name=L + "mx", tag="bigA")
                mn2 = sc(L + "mn2")
                for t in range(NT):
                    hT = htp.tile([128, 2, 128], F32, name="hT", tag="hT")
                    for kc in range(2):
                        psT = pst.tile([128, 128], F32, name="psT", tag="psT")
                        nc.tensor.transpose(
                            psT[:], h_in[:, t, kc * 128 : (kc + 1) * 128], ident[:])
                        nc.vector.tensor_copy(hT[:, kc, :], psT[:])
                    pmx = psmx.tile([128, D], F32, name="pmx", tag="pmx")
                    nc.tensor.matmul(pmx[:], hT[:, 0, :], w[:, 0, :],
                                     start=True, stop=False)
                    nc.tensor.matmul(pmx[:], hT[:, 1, :], w[:, 1, :],
                                     start=False, stop=True)
                    square_accum(pmx[:], mn2[:, t : t + 1])
                    nc.vector.tensor_copy(mx_all[:, t, :], pmx[:])

                mxn = sc(L + "mxn")
                sqrt_act(mxn, mn2, L + "mxn")
                nc.vector.tensor_scalar_max(mxn[:], mxn[:], MINN)
                rxn = sc(L + "rxn")
                clamp_recip(rxn, hnorm, L + "rxn")
                rmxn = sc(L + "rmxn")
                nc.vector.reciprocal(rmxn[:], mxn[:])
                atx = sc(L + "atx")
                artanh_ln(atx, hnorm, L + "atx")
                targ = sc(L + "targ")
                nc.vector.tensor_tensor(targ[:], mxn[:], rxn[:], ALU.mult)
                nc.vector.tensor_tensor(targ[:], targ[:], atx[:], ALU.mult)
                th = sc(L + "th")
                tanh_act(th, targ, L + "th", pre=0.5)
                sres = sc(L + "sres")
                nc.vector.tensor_tensor(sres[:], th[:], rmxn[:], ALU.mult)
                rth = sc(L + "rth")
                clamp_recip(rth, th, L + "rth")
                f1 = sc(L + "f1")
                nc.vector.tensor_scalar(f1[:], rth[:], MAXN, 1.0, ALU.mult, ALU.min)
                nres = sc(L + "nres")
                nc.vector.tensor_scalar_min(nres[:], th[:], MAXN)
                x2 = sc(L + "x2")
                nc.vector.tensor_tensor(x2[:], nres[:], nres[:], ALU.mult)

                ryp = sc(L + "ryp")
                for t in range(NT):
                    prod = sqp.tile([128, D], F32, name="prodt", tag="prodt")
                    nc.vector.tensor_tensor(prod[:], mx_all[:, t, :], hb[:],
                                            ALU.mult)
                    nc.vector.reduce_sum(ryp[:, t : t + 1], prod[:],
                                         axis=mybir.AxisListType.X)

                xy = sc(L + "xy")
                nc.vector.tensor_tensor(xy[:], ryp[:], sres[:], ALU.mult)
                nc.vector.tensor_tensor(xy[:], xy[:], f1[:], ALU.mult)
                apre = sc(L + "apre")
                nc.vector.tensor_scalar(apre[:], xy[:], 2.0, 1.0 + y2,
                                        ALU.mult, ALU.add)
                alpha = sc(L + "alpha")
                nc.vector.tensor_tensor(alpha[:], apre[:], f1[:], ALU.mult)
                beta = sc(L + "beta")
                nc.scalar.activation(beta[:], x2[:], AF.Identity,
                                     bias=1.0, scale=-1.0)
                den = sc(L + "den")
                nc.vector.tensor_scalar(den[:], x2[:], y2, 1.0, ALU.mult, ALU.add)
                xy2 = sc(L + "xy2")
                nc.vector.tensor_scalar_mul(xy2[:], xy[:], 2.0)
                nc.vector.tensor_tensor(den[:], den[:], xy2[:], ALU.add)
                dinv = sc(L + "dinv")
                clamp_recip(dinv, den, L + "dinv")
                asc = sc(L + "asc")
                nc.vector.tensor_tensor(asc[:], alpha[:], dinv[:], ALU.mult)
                nc.vector.tensor_tensor(asc[:], asc[:], sres[:], ALU.mult)
                bsc = sc(L + "bsc")
                nc.vector.tensor_tensor(bsc[:], beta[:], dinv[:], ALU.mult)

                h2_all = persist.tile([128, NT, D], F32, name=L + "h2", tag="bigB")
                hn2 = sc(L + "hn2")
                for t in range(NT):
                    t1 = sqp.tile([128, D], F32, name="t1t", tag="t1t")
                    nc.vector.tensor_scalar_mul(t1[:], mx_all[:, t, :],
                                                asc[:, t : t + 1])
                    t2 = sqp.tile([128, D], F32, name="t2t", tag="t2t")
                    nc.scalar.activation(t2[:], hb[:], AF.Copy,
                                         scale=bsc[:, t : t + 1])
                    nc.vector.tensor_tensor(h2_all[:, t, :], t1[:], t2[:], ALU.add)
                    square_accum(h2_all[:, t, :], hn2[:, t : t + 1])

                hn = sc(L + "hn")
                sqrt_act(hn, hn2, L + "hn")
                rhn = sc(L + "rhn")
                clamp_recip(rhn, hn, L + "rhn")
                f2 = sc(L + "f2")
                nc.vector.tensor_scalar(f2[:], rhn[:], MAXN, 1.0, ALU.mult, ALU.min)
                m = sc(L + "m")
                nc.vector.tensor_scalar_min(m[:], hn[:], MAXN)
                rm = sc(L + "rm")
                clamp_recip(rm, m, L + "rm")
                atm = sc(L + "atm")
                artanh_ln(atm, m, L + "atm")
                g = sc(L + "g")
                nc.vector.tensor_tensor(g[:], atm[:], rm[:], ALU.mult)
                nc.vector.tensor_tensor(g[:], g[:], f2[:], ALU.mult)
                nc.vector.tensor_scalar_mul(g[:], g[:], 0.5)

                xt_all = persist.tile([128, NT, D], BF16, name=L + "xt",
                                      tag="bigC")
                for t in range(NT):
                    nc.vector.tensor_scalar_mul(xt_all[:, t, :], h2_all[:, t, :],
                                                g[:, t : t + 1])
                return xt_all

            def spmm_layer(li, xt_all, e_out_d):
                """Chunked partial spmm + ReduceScatter + per-chunk HypAct.

                Queue discipline (all engine queues are in-order, so no
                pre-RS work may be emitted after RS-dependent work on the
                same queue): matmuls on tensor; PSUM evacuation on vector
                BEFORE any act-chain ops of the same cycle (act chains are
                emitted LAG chunks behind); A-stream + stage-out DMAs on
                sync (never RS-dependent); RS + its readback on gpsimd;
                e-store issued from vector right after its act chain.
                Returns (e_all [128,NT,D] f32, mm2_all [128,NT] norms)."""
                L = f"l{li}_"
                LAG = 2
                e_all = persist.tile([128, NT, D], F32, name=L + "e", tag="bigE")
                xt2_all = persist.tile([128, NT, D], F32, name=L + "xt2",
                                       tag="bigD")
                mm2_all = sc(L + "mm2")
                aggs = {}

                def hyp_act_chunk(p):
                    P = f"{L}c{p}_"
                    agg = aggs.pop(p)
                    r2 = sc(P + "r2", CT)
                    for i in range(CT):
                        square_accum(agg[:, i, :], r2[:, i : i + 1])
                    rn = sc(P + "rn", CT)
                    sqrt_act(rn, r2, P + "rn2", CT)
                    rrn = sc(P + "rrn", CT)
                    clamp_recip(rrn, rn, P + "rrn", CT)
                    th2 = sc(P + "th2", CT)
                    tanh_act(th2, rn, P + "th2", CT)
                    m1 = sc(P + "m1", CT)
                    nc.vector.tensor_scalar_min(m1[:], th2[:], MAXN)
                    rm1 = sc(P + "rm1", CT)
                    clamp_recip(rm1, m1, P + "rm1", CT)
                    s1 = sc(P + "s1", CT)
                    nc.vector.tensor_tensor(s1[:], m1[:], rrn[:], ALU.mult)
                    atq = sc(P + "atq", CT)
                    artanh_ln(atq, m1, P + "atq", CT)
                    qs = sc(P + "qs", CT)
                    nc.vector.tensor_tensor(qs[:], s1[:], atq[:], ALU.mult)
                    nc.vector.tensor_tensor(qs[:], qs[:], rm1[:], ALU.mult)
                    nc.vector.tensor_scalar_mul(qs[:], qs[:], 0.5)

                    n2b = sc(P + "n2b", CT)
                    for i in range(CT):
                        t = p * CT + i
                        nc.vector.tensor_scalar(xt2_all[:, t, :], agg[:, i, :],
                                                qs[:, i : i + 1], 0.0,
                                                ALU.mult, ALU.max)
                        square_accum(xt2_all[:, t, :], n2b[:, i : i + 1])

                    un2 = sc(P + "un2", CT)
                    sqrt_act(un2, n2b, P + "un2", CT)
                    run2 = sc(P + "run2", CT)
                    clamp_recip(run2, un2, P + "run2", CT)
                    th3 = sc(P + "th3", CT)
                    tanh_act(th3, un2, P + "th3", CT)
                    mm2 = mm2_all[:, p * CT : (p + 1) * CT]
                    nc.vector.tensor_scalar_min(mm2, th3[:], MAXN)
                    ss = sc(P + "ss", CT)
                    nc.vector.tensor_tensor(ss[:], mm2, run2[:], ALU.mult)

                    for i in range(CT):
                        t = p * CT + i
                        nc.vector.tensor_scalar_mul(e_all[:, t, :],
                                                    xt2_all[:, t, :],
                                                    ss[:, i : i + 1])
                    nc.scalar.dma_start(
                        e_out_d.ap()[p * CT : (p + 1) * CT].rearrange(
                            "t p d -> p t d"),
                        e_all[:, p * CT : (p + 1) * CT, :])

                a_tiles = {}
                a_tiles[0] = ablk.tile([128, NT, CHC], BF16, name="a_sb",
                                       tag="a_sb")
                nc.sync.dma_start(a_tiles[0][:], a_d.ap()[0])
                for p in range(NCH):
                    P = f"{L}c{p}_"
                    if p + 1 < NCH:
                        a_tiles[p + 1] = ablk.tile([128, NT, CHC], BF16,
                                                   name="a_sb", tag="a_sb")
                        nc.sync.dma_start(a_tiles[p + 1][:], a_d.ap()[p + 1])
                    a_sb = a_tiles.pop(p)
                    # rs_in is rank-major [NCORES, 128, CT*D]; pair pr of the
                    # chunk IS rank pr's slice (CT=2), written per-pair as
                    # soon as its matmuls finish (contiguous 1KB runs).
                    rs_in = dram.tile([NCORES, 128, CT * D], BF16,
                                      name=P + "rsi", tag=f"rsi{li}_{p}")
                    for pr in range(CHT // 2):
                        ps = psp.tile([128, 2, D], F32, name="ps", tag="ps")
                        for kt in range(NT):
                            for h in range(2):
                                nc.tensor.matmul(
                                    ps[:, h, :],
                                    a_sb[:, kt,
                                         pr * 256 + h * 128 : pr * 256 + h * 128 + 128],
                                    xt_all[:, kt, :],
                                    start=(kt == 0 and h == 0),
                                    stop=(kt == NT - 1),
                                    skip_group_check=True)
                        pair = stg.tile([128, CT, D], BF16, name="pair",
                                        tag="pair")
                        nc.vector.tensor_copy(pair[:], ps[:])
                        nc.sync.dma_start(
                            rs_in[:][pr],
                            pair[:].rearrange("p t d -> p (t d)"))
                    rs_out = dram.tile([128, CT * D], BF16, name=P + "rso",
                                       tag=f"rso{li}_{p}")
                    nc.gpsimd.collective_compute(
                        "ReduceScatter", ALU.add, replica_groups=groups,
                        ins=[rs_in[:].opt()], outs=[rs_out[:].opt()])
                    agg = aggp.tile([128, CT, D], BF16, name="agg", tag="agg")
                    nc.gpsimd.dma_start(
                        agg[:].rearrange("p t d -> p (t d)"), rs_out[:])
                    aggs[p] = agg
                    if p >= LAG:
                        hyp_act_chunk(p - LAG)
                for p in range(NCH - LAG, NCH):
                    hyp_act_chunk(p)
                return e_all, mm2_all

            xt0 = hyp_linear(0, h_all, mn0)
            e1_all, n1 = spmm_layer(0, xt0, e1_d)
            xt1 = hyp_linear(1, e1_all, n1)
            spmm_layer(1, xt1, e2_d)

    nc.compile()
    return nc


def _hyp_bias(b):
    """proj(expmap0(b, c=1), c=1) in float32, mirroring the reference."""
    b = b.astype(np.float32)
    un = np.maximum(np.sqrt((b * b).sum()), np.float32(MINN)).astype(np.float32)
    h = (np.tanh(un) * b / un).astype(np.float32)
    n = np.maximum(np.sqrt((h * h).sum()), np.float32(MINN)).astype(np.float32)
    if n > np.float32(MAXN):
        h = (h / n * np.float32(MAXN)).astype(np.float32)
    return h


def prepare_inputs(x, W1, b1, W2, b2, edge_index, edge_weight):
    mmnp = _mm_np_dtype()
    x = np.asarray(x, np.float32)
    W1 = np.asarray(W1, np.float32)
    W2 = np.asarray(W2, np.float32)
    b1 = np.asarray(b1, np.float32)
    b2 = np.asarray(b2, np.float32)
    ew = np.asarray(edge_weight, np.float32)
    src = np.asarray(edge_index[0], np.int64)
    dst = np.asarray(edge_index[1], np.int64)

    # A[dst, src]; device matmul computes partial[dst] += A^T[src, dst] rows
    AT = np.zeros((NP, NP), np.float32)
    np.add.at(AT, (src, dst), ew)
    ATb = AT.astype(mmnp)

    xfull = np.zeros((NP, D), np.float32)
    xfull[:N] = x

    hb1 = _hyp_bias(b1)
    hb2 = _hyp_bias(b2)
    y2s = (float((hb1.astype(np.float64) ** 2).sum()),
           float((hb2.astype(np.float64) ** 2).sum()))

    w1t = np.ascontiguousarray(W1.T).reshape(2, 128, D)
    w2t = np.ascontiguousarray(W2.T).reshape(2, 128, D)
    hb1_b = np.tile(hb1[None, :], (128, 1)).astype(np.float32)
    hb2_b = np.tile(hb2[None, :], (128, 1)).astype(np.float32)

    in_maps = []
    for c in range(NCORES):
        tl = own_tiles(c)
        xcr = np.stack([xfull[g * 128 : (g + 1) * 128] for g in tl], 0)
        # a[p, s, j, :] = AT[tl[j]*128 + s, p*CHC:(p+1)*CHC]
        ac = np.empty((NCH, 128, NT, CHC), mmnp)
        for j, g in enumerate(tl):
            rows = ATb[g * 128 : (g + 1) * 128]  # [128, NP]
            for p in range(NCH):
                ac[p, :, j, :] = rows[:, p * CHC : (p + 1) * CHC]
        in_maps.append({
            "xc": np.ascontiguousarray(xcr),
            "a": ac,
            "w1t": w1t, "w2t": w2t,
            "hb1": hb1_b, "hb2": hb2_b,
        })
    return in_maps, y2s


def assemble(results):
    e1 = np.zeros((NP, D), np.float32)
    e2 = np.zeros((NP, D), np.float32)
    for c, r in enumerate(results):
        tl = own_tiles(c)
        r1 = r["e1"].reshape(NT, 128, D)
        r2 = r["e2"].reshape(NT, 128, D)
        for j, g in enumerate(tl):
            e1[g * 128 : (g + 1) * 128] = r1[j]
            e2[g * 128 : (g + 1) * 128] = r2[j]
    return np.stack([e1[:N], e2[:N]], 0).astype(np.float32)


def run(inputs, trace=False):
    in_maps, y2s = prepare_inputs(**inputs)
    nc = build_nc(y2s)
    res = run_bass_kernel_spmd(nc, in_maps, core_ids=list(range(NCORES)),
                               trace=trace)
    return assemble(res.results), res


def kernel(**inputs):
    out, _ = run(inputs, trace=False)
    return out
